# revision 3
# baseline (speedup 1.0000x reference)
"""Causal attention (B=4, S=2048, D=1024) on 8 Trainium2 NeuronCores.

Sharding: data-parallel over batch (4) x query-block-parallel (2 cores per
batch).  Global q-tiles (128 rows each, 16 per batch) are dealt round-robin:
core h=0 of a pair takes even tiles, h=1 odd tiles.  Each core loads ONLY its
own-parity x rows (xq); the transposed xq tiles feed the Q projection AND the
K/V projections for the core's own-parity key tiles.  The halves are then
exchanged with a pair AllGather (HBM->HBM) whose latency is hidden behind the
Q projection; both cores read BOTH gather slots back into a parity-blocked
SBUF layout (slot p = parity p on every core), keeping the instruction stream
SPMD.  The causal asymmetry between the two cores lives entirely in a per-core
additive-mask input, exactly as in the duplicated-projection variant.

All matmuls run in bf16 with fp32 PSUM accumulation:
  xqT     : PE-transposed activations (d on partitions)
  QT[e,q] = wq^T xq^T / sqrt(D), KTo[e,ko] = wk^T xq^T, Vo[ko,e] = xq wv
  (KTo/Vo cover the 8 own-parity key tiles; AllGather -> kT/v both parities)
  S[q,k]  = QT^T KT per parity block (chunks of <=512 cols in PSUM), the
            block-boundary tile gets the additive mask half for that parity
  P       = exp(S) (scores are O(1) -- max-subtraction is unnecessary),
            fused row-sum via activation accum_out
  O[q,e]  = (P^T)^T V accumulated over both parity blocks' 128-key tiles,
            scaled by 1/rowsum
"""

import os

os.environ.setdefault("MYCRO_LOCAL_CACHE", "1")

import numpy as np

import concourse.bacc as bacc
import concourse.tile as tile
from concourse import mybir
from concourse.bass_utils import run_bass_kernel_spmd
from concourse.masks import make_identity

B, S, D = 4, 2048, 1024
P = 128
QL = S // 2          # queries per core == own-parity keys per core
NCORES = 8
DT = D // P          # 8 d-tiles (contraction)
ET = D // P          # 8 e-tiles
NQT = QL // P        # 8 q-tiles per core
NKT = QL // P        # 8 own-parity k-tiles per core
F32 = mybir.dt.float32
BF16 = mybir.dt.bfloat16
NEG = -30000.0       # additive mask value; exp() underflows to exactly 0
PAIRS = [[2 * b, 2 * b + 1] for b in range(B)]


def _body(tc, xq, wq, wk, wv, mask, out):
    nc = tc.nc
    with (
        tc.tile_pool(name="consts", bufs=1) as consts,
        tc.tile_pool(name="qkv", bufs=1) as qkv,
        tc.tile_pool(name="dram", bufs=1, space="DRAM") as dram,
    ):
        ident = consts.tile([P, P], BF16)
        make_identity(nc, ident)
        mask_sb = consts.tile([P, 256], F32)
        nc.sync.dma_start(mask_sb, mask)

        qT = qkv.tile([P, ET, QL], BF16)        # [e_in, e_tile, q]
        kT = qkv.tile([P, ET, 2, QL], BF16)     # [e_in, e_tile, parity, k]
        v = qkv.tile([P, 2, NKT, D], BF16)      # [k_in, parity, k_tile, e]

        # HBM bounce buffers for the pair exchange.  Section 0..7 = KT^own
        # (e-tile major), 8..15 = V^own (k-tile major).
        kv_loc = dram.tile([2 * NKT, P, D], BF16)
        kv_gth = dram.tile([2, 2 * NKT, P, D], BF16)

        # ------------------------------ projections ------------------------
        outer = tc.tile_pool(name="pmm", bufs=4, space="PSUM")
        pmm = outer.__enter__()
        with (
            tc.tile_pool(name="wsb", bufs=2) as wpool,
            tc.tile_pool(name="wqp", bufs=1) as wqp,
            tc.tile_pool(name="stage", bufs=4) as stpool,
            tc.tile_pool(name="castq", bufs=3) as castq,
            tc.tile_pool(name="kvout", bufs=4) as kvout,
            tc.tile_pool(name="xqp", bufs=1) as xqp,
            tc.tile_pool(name="ptr", bufs=4, space="PSUM") as ptr,
        ):
            def load_weight(w_ap, pool):
                wsb = pool.tile([P, DT, D], BF16, tag="w")
                for d in range(DT):
                    stg = stpool.tile([P, D], F32, tag="stage")
                    nc.sync.dma_start(stg, w_ap[d * P:(d + 1) * P, :])
                    nc.vector.tensor_copy(wsb[:, d, :], stg)
                return wsb

            # ---- load + transpose own-parity x tiles
            wk_sb = None
            xqT = xqp.tile([P, DT, QL], BF16)
            for s in range(NQT):
                stg = stpool.tile([P, D], F32, tag="stage")
                xb = castq.tile([P, D], BF16, tag="cast")
                if s == 0:
                    for h_ in range(2):
                        cols = slice(h_ * (D // 2), (h_ + 1) * (D // 2))
                        nc.sync.dma_start(stg[:, cols],
                                          xq[s * P:(s + 1) * P, cols])
                        nc.vector.tensor_copy(xb[:, cols], stg[:, cols])
                else:
                    nc.sync.dma_start(stg, xq[s * P:(s + 1) * P, :])
                    nc.vector.tensor_copy(xb, stg)
                for d in range(DT):
                    pst = ptr.tile([P, P], BF16, tag="tp")
                    nc.tensor.transpose(pst, xb[:, d * P:(d + 1) * P], ident)
                    nc.vector.tensor_copy(xqT[:, d, s * P:(s + 1) * P], pst)
                if s == 1:
                    wk_sb = load_weight(wk, wpool)

            # ---- K^own: KT[e, ko] = wk^T xq^T  (own-parity keys)
            wv_sb = None
            for c in range(QL // 512):
                for e in range(ET):
                    ps = pmm.tile([P, 512], F32, tag="mm")
                    for d in range(DT):
                        nc.tensor.matmul(
                            ps, wk_sb[:, d, e * P:(e + 1) * P],
                            xqT[:, d, c * 512:(c + 1) * 512],
                            start=(d == 0), stop=(d == DT - 1))
                    ksb = kvout.tile([P, 512], BF16, tag="kv")
                    nc.scalar.copy(ksb, ps)
                    nc.sync.dma_start(kv_loc[e, :, c * 512:(c + 1) * 512], ksb)
                if c == 0:
                    wv_sb = load_weight(wv, wpool)

            # ---- V^own: V[ko, e] = xq wv
            wq_sb = None
            for k in range(NKT):
                for ec in range(D // 512):
                    ps = pmm.tile([P, 512], F32, tag="mm")
                    for d in range(DT):
                        nc.tensor.matmul(
                            ps, xqT[:, d, k * P:(k + 1) * P],
                            wv_sb[:, d, ec * 512:(ec + 1) * 512],
                            start=(d == 0), stop=(d == DT - 1))
                    vsb = kvout.tile([P, 512], BF16, tag="kv")
                    nc.scalar.copy(vsb, ps)
                    nc.sync.dma_start(
                        kv_loc[NKT + k, :, ec * 512:(ec + 1) * 512], vsb)
                if k == 0:
                    wq_sb = load_weight(wq, wqp)

            # ---- pair exchange: slot p of kv_gth = parity-p core's half
            nc.gpsimd.collective_compute(
                "AllGather",
                mybir.AluOpType.bypass,
                replica_groups=PAIRS,
                ins=[kv_loc.opt()],
                outs=[kv_gth.opt()],
            )

            # ---- Q projection (overlaps the collective)
            for c in range(QL // 512):
                for e in range(ET):
                    ps = pmm.tile([P, 512], F32, tag="mm")
                    for d in range(DT):
                        nc.tensor.matmul(
                            ps, wq_sb[:, d, e * P:(e + 1) * P],
                            xqT[:, d, c * 512:(c + 1) * 512],
                            start=(d == 0), stop=(d == DT - 1))
                    nc.scalar.mul(qT[:, e, c * 512:(c + 1) * 512], ps,
                                  1.0 / 32.0)

            # ---- read back both parity halves (own half comes back too --
            # uniform addressing keeps the program SPMD)
            for p in range(2):
                for e in range(ET):
                    nc.sync.dma_start(kT[:, e, p, :], kv_gth[p, e, :, :])
            for p in range(2):
                for k in range(NKT):
                    nc.sync.dma_start(v[:, p, k, :], kv_gth[p, NKT + k, :, :])

        # ------------------------------ attention --------------------------
        with (
            tc.tile_pool(name="attn", bufs=3) as apool,
            tc.tile_pool(name="ptsb", bufs=6) as ptpool,
            tc.tile_pool(name="stats", bufs=2) as spool,
            tc.tile_pool(name="psT", bufs=2, space="PSUM") as psT,
            tc.tile_pool(name="psO", bufs=1, space="PSUM") as psO,
        ):
            psS = pmm
            for j in (7, 6, 5, 4, 3, 2, 1, 0):
                w = (j + 1) * P              # cols per parity block
                p_sb = apool.tile([P, 2, QL], BF16, tag="p")
                lsum = spool.tile([P, 2, NQT], F32, tag="lsum")
                for p in range(2):
                    off = 0
                    while off < w:
                        cw = min(512, w - off)
                        ps = psS.tile([P, cw], F32, tag="mm")
                        for e in range(ET):
                            nc.tensor.matmul(
                                ps, qT[:, e, j * P:(j + 1) * P],
                                kT[:, e, p, off:off + cw],
                                start=(e == 0), stop=(e == ET - 1))
                        if off + cw == w:
                            nc.vector.tensor_add(
                                ps[:, cw - P:cw], ps[:, cw - P:cw],
                                mask_sb[:, p * P:(p + 1) * P])
                        # 128-wide exp subtiles: each P^T transpose can start
                        # as soon as its own columns are exponentiated
                        for si in range(cw // P):
                            col = off + si * P
                            nc.scalar.activation(
                                p_sb[:, p, col:col + P],
                                ps[:, si * P:(si + 1) * P],
                                mybir.ActivationFunctionType.Exp,
                                accum_out=lsum[:, p, col // P:col // P + 1])
                        off += cw
                l2 = spool.tile([P, 2], F32, tag="l2")
                nc.vector.reduce_sum(l2, lsum[:, :, 0:j + 1],
                                     axis=mybir.AxisListType.X)
                l_ = spool.tile([P, 1], F32, tag="l")
                nc.vector.tensor_add(l_, l2[:, 0:1], l2[:, 1:2])
                linv = spool.tile([P, 1], F32, tag="linv")
                nc.vector.reciprocal(linv, l_)

                po = psO.tile([P, D], F32, tag="o")
                nk = 2 * (j + 1)
                for ki in range(nk):
                    p, k = ki % 2, ki // 2
                    pt_ps = psT.tile([P, P], BF16, tag="pt")
                    nc.tensor.transpose(pt_ps, p_sb[:, p, k * P:(k + 1) * P],
                                        ident)
                    pt = ptpool.tile([P, P], BF16, tag="ptsb")
                    nc.vector.tensor_copy(pt, pt_ps)
                    for c in range(D // 512):
                        nc.tensor.matmul(
                            po[:, c * 512:(c + 1) * 512], pt,
                            v[:, p, k, c * 512:(c + 1) * 512],
                            start=(ki == 0), stop=(ki == nk - 1))
                o_sb = apool.tile([P, D], F32, tag="o")
                for c in range(D // 512):
                    nc.vector.tensor_scalar_mul(
                        o_sb[:, c * 512:(c + 1) * 512],
                        po[:, c * 512:(c + 1) * 512], linv)
                nc.sync.dma_start(out[j * P:(j + 1) * P, :], o_sb)
        outer.__exit__(None, None, None)


_PROG = None


def _get_prog():
    global _PROG
    if _PROG is None:
        nc = bacc.Bacc("TRN2", target_bir_lowering=False, debug=False,
                       enable_asserts=False)
        xq = nc.dram_tensor("xq", (QL, D), F32, kind="ExternalInput").ap()
        wq = nc.dram_tensor("wq", (D, D), F32, kind="ExternalInput").ap()
        wk = nc.dram_tensor("wk", (D, D), F32, kind="ExternalInput").ap()
        wv = nc.dram_tensor("wv", (D, D), F32, kind="ExternalInput").ap()
        mask = nc.dram_tensor("mask", (P, 256), F32, kind="ExternalInput").ap()
        out = nc.dram_tensor("out", (QL, D), F32, kind="ExternalOutput").ap()
        with tile.TileContext(nc) as tc:
            _body(tc, xq, wq, wk, wv, mask, out)
        nc.compile()
        _PROG = nc
    return _PROG


def _mask_np(h):
    r = np.arange(P)[:, None]
    c = np.arange(P)[None, :]
    tri = np.where(c <= r, 0.0, NEG).astype(np.float32)
    m = np.zeros((P, 256), np.float32)
    if h == 0:
        m[:, :P] = tri
        m[:, P:] = NEG
    else:
        m[:, P:] = tri
    return m


def _in_map_for_core(inputs, core):
    b, h = core // 2, core % 2
    xb = np.asarray(inputs["x"], np.float32)[b]
    xqb = np.ascontiguousarray(xb.reshape(NQT, 2, P, D)[:, h].reshape(QL, D))
    return {
        "xq": xqb,
        "wq": np.ascontiguousarray(np.asarray(inputs["wq"], np.float32)),
        "wk": np.ascontiguousarray(np.asarray(inputs["wk"], np.float32)),
        "wv": np.ascontiguousarray(np.asarray(inputs["wv"], np.float32)),
        "mask": _mask_np(h),
    }


def _run(inputs, trace=False, tmpdir=None):
    nc = _get_prog()
    in_maps = [_in_map_for_core(inputs, c) for c in range(NCORES)]
    try:
        res = run_bass_kernel_spmd(nc, in_maps, core_ids=list(range(NCORES)),
                                   trace=trace, tmpdir=tmpdir)
    except Exception:
        # first execution of a fresh NEFF occasionally trips a transient
        # device error on this stack; one retry has always succeeded
        res = run_bass_kernel_spmd(nc, in_maps, core_ids=list(range(NCORES)),
                                   trace=trace, tmpdir=tmpdir)
    outf = np.empty((B, S, D), np.float32)
    for core in range(NCORES):
        b, h = core // 2, core % 2
        o = np.asarray(res.results[core]["out"], np.float32)
        outf[b].reshape(NQT, 2, P, D)[:, h] = o.reshape(NQT, P, D)
    return outf, res


def kernel(x, wq, wk, wv):
    outf, _ = _run({"x": x, "wq": wq, "wk": wk, "wv": wv}, trace=False)
    return outf


# revision 4
# speedup vs baseline: 1.1291x; 1.1291x over previous
"""Causal attention (B=4, S=2048, D=1024) on 8 Trainium2 NeuronCores.

Sharding: data-parallel over batch (4) x query-block-parallel (2 cores per
batch).  Global q-tiles (128 rows each, 16 per batch) are dealt round-robin:
core h=0 of a pair takes even tiles, h=1 odd tiles.

The K projection is split across the pair: each core computes K^T only for
its own-parity key tiles (which are exactly its own q rows, so the transposed
xq tiles feed the Q projection AND the half-K projection), then a 2MB pair
AllGather (HBM->HBM) exchanges the halves while the core computes the full V
projection and the Q projection.  Both cores read BOTH gather slots back into
a parity-blocked kT layout (slot p = parity p on every core), keeping the
instruction stream SPMD.  V is computed duplicated from the full x (a 4MB
V-exchange does not fit in the collective's latency budget; a 2MB one does).
The causal asymmetry between the two cores lives in a per-core additive-mask
input.

All matmuls run in bf16 with fp32 PSUM accumulation:
  xqT/xT  : PE-transposed activations (d on partitions)
  QT[e,q] = wq^T xq^T / sqrt(D), KTo[e,ko] = wk^T xq^T, V[k,e] = x wv
  S[q,k]  = QT^T KT per parity block (chunks of <=512 cols in PSUM), the
            block-boundary tile gets the additive mask half for that parity
  P       = exp(S) (scores are O(1) -- max-subtraction is unnecessary),
            fused row-sum via activation accum_out
  O[q,e]  = (P^T)^T V accumulated over both parity blocks' 128-key tiles,
            scaled by 1/rowsum
"""

import os

os.environ.setdefault("MYCRO_LOCAL_CACHE", "1")

import numpy as np

import concourse.bacc as bacc
import concourse.tile as tile
from concourse import mybir
from concourse.bass_utils import run_bass_kernel_spmd
from concourse.masks import make_identity

B, S, D = 4, 2048, 1024
P = 128
QL = S // 2          # queries per core == own-parity keys per core
NCORES = 8
DT = D // P          # 8 d-tiles (contraction)
ET = D // P          # 8 e-tiles
ST = S // P          # 16 s-tiles
NQT = QL // P        # 8 q-tiles per core
NKT = QL // P        # 8 own-parity k-tiles per core
F32 = mybir.dt.float32
BF16 = mybir.dt.bfloat16
NEG = -30000.0       # additive mask value; exp() underflows to exactly 0
PAIRS = [[2 * b, 2 * b + 1] for b in range(B)]


def _body(tc, x, xq, wq, wk, wv, mask, out):
    nc = tc.nc
    with (
        tc.tile_pool(name="consts", bufs=1) as consts,
        tc.tile_pool(name="qkv", bufs=1) as qkv,
        tc.tile_pool(name="dram", bufs=1, space="DRAM") as dram,
    ):
        ident = consts.tile([P, P], BF16)
        make_identity(nc, ident)
        mask_sb = consts.tile([P, 256], F32)
        nc.sync.dma_start(mask_sb, mask)

        qT = qkv.tile([P, ET, QL], BF16)        # [e_in, e_tile, q]
        kT = qkv.tile([P, ET, 2, QL], BF16)     # [e_in, e_tile, parity, k]
        v = qkv.tile([P, ST, D], BF16)          # [k_in, global k_tile, e]

        # HBM bounce buffers for the pair K exchange (e-tile major).
        k_loc = dram.tile([ET, P, QL], BF16)
        k_gth = dram.tile([2, ET, P, QL], BF16)

        # ------------------------------ projections ------------------------
        outer = tc.tile_pool(name="pmm", bufs=4, space="PSUM")
        pmm = outer.__enter__()
        with (
            tc.tile_pool(name="wsb", bufs=2) as wpool,
            tc.tile_pool(name="stage", bufs=4) as stpool,
            tc.tile_pool(name="castq", bufs=3) as castq,
            tc.tile_pool(name="castx", bufs=6) as castx,
            tc.tile_pool(name="kvout", bufs=4) as kvout,
            tc.tile_pool(name="xqp", bufs=1) as xqp,
            tc.tile_pool(name="xtp", bufs=1) as xtp,
            tc.tile_pool(name="ptr", bufs=4, space="PSUM") as ptr,
        ):
            def load_weight(w_ap):
                wsb = wpool.tile([P, DT, D], BF16, tag="w")
                for d in range(DT):
                    stg = stpool.tile([P, D], F32, tag="stage")
                    nc.sync.dma_start(stg, w_ap[d * P:(d + 1) * P, :])
                    nc.vector.tensor_copy(wsb[:, d, :], stg)
                return wsb

            def load_cast(x_ap, s, cpool, split=False):
                stg = stpool.tile([P, D], F32, tag="stage")
                xb = cpool.tile([P, D], BF16, tag="cast")
                if split:
                    for h_ in range(2):
                        cols = slice(h_ * (D // 2), (h_ + 1) * (D // 2))
                        nc.sync.dma_start(stg[:, cols],
                                          x_ap[s * P:(s + 1) * P, cols])
                        nc.vector.tensor_copy(xb[:, cols], stg[:, cols])
                else:
                    nc.sync.dma_start(stg, x_ap[s * P:(s + 1) * P, :])
                    nc.vector.tensor_copy(xb, stg)
                return xb

            def transpose_into(xb, s, dst):
                for d in range(DT):
                    pst = ptr.tile([P, P], BF16, tag="tp")
                    nc.tensor.transpose(pst, xb[:, d * P:(d + 1) * P], ident)
                    nc.vector.tensor_copy(dst[:, d, s * P:(s + 1) * P], pst)

            # ---- load + transpose own-parity rows (xq); prefetch x
            x_pref = {}
            wk_sb = None
            xqT = xqp.tile([P, DT, QL], BF16)
            for s in range(NQT):
                xb = load_cast(xq, s, castq, split=(s == 0))
                transpose_into(xb, s, xqT)
                if s == 1:
                    wk_sb = load_weight(wk)
                if s == 3:
                    for sp in range(2):
                        x_pref[sp] = load_cast(x, sp, castx)
                if s == 5:
                    for sp in range(2, 4):
                        x_pref[sp] = load_cast(x, sp, castx)

            # ---- K^own: KT[e, ko] = wk^T xq^T  (own-parity keys)
            wv_sb = None
            for c in range(QL // 512):
                for e in range(ET):
                    ps = pmm.tile([P, 512], F32, tag="mm")
                    for d in range(DT):
                        nc.tensor.matmul(
                            ps, wk_sb[:, d, e * P:(e + 1) * P],
                            xqT[:, d, c * 512:(c + 1) * 512],
                            start=(d == 0), stop=(d == DT - 1))
                    ksb = kvout.tile([P, 512], BF16, tag="kv")
                    nc.scalar.copy(ksb, ps)
                    nc.sync.dma_start(k_loc[e, :, c * 512:(c + 1) * 512], ksb)
                if c == 0:
                    wv_sb = load_weight(wv)

            # ---- pair exchange: slot p of k_gth = parity-p core's K half
            nc.gpsimd.collective_compute(
                "AllGather",
                mybir.AluOpType.bypass,
                replica_groups=PAIRS,
                ins=[k_loc.opt()],
                outs=[k_gth.opt()],
            )

            # ---- V (duplicated, global key order) + Q, overlapping the cc
            wq_sb = None
            xT = xtp.tile([P, DT, S], BF16)
            for c in range(S // 512):
                for s in range(4 * c, 4 * c + 4):
                    xb = x_pref.pop(s, None)
                    if xb is None:
                        xb = load_cast(x, s, castx)
                    transpose_into(xb, s, xT)
                for k in range(4 * c, 4 * c + 4):
                    for ec in range(D // 512):
                        ps = pmm.tile([P, 512], F32, tag="mm")
                        for d in range(DT):
                            nc.tensor.matmul(
                                ps, xT[:, d, k * P:(k + 1) * P],
                                wv_sb[:, d, ec * 512:(ec + 1) * 512],
                                start=(d == 0), stop=(d == DT - 1))
                        nc.scalar.copy(v[:, k, ec * 512:(ec + 1) * 512], ps)
                if c == 0:
                    wq_sb = load_weight(wq)

            # ---- Q projection
            for c in range(QL // 512):
                for e in range(ET):
                    ps = pmm.tile([P, 512], F32, tag="mm")
                    for d in range(DT):
                        nc.tensor.matmul(
                            ps, wq_sb[:, d, e * P:(e + 1) * P],
                            xqT[:, d, c * 512:(c + 1) * 512],
                            start=(d == 0), stop=(d == DT - 1))
                    nc.scalar.mul(qT[:, e, c * 512:(c + 1) * 512], ps,
                                  1.0 / 32.0)

            # ---- read back both K parity halves (own half comes back too --
            # uniform addressing keeps the program SPMD)
            for p in range(2):
                for e in range(ET):
                    nc.sync.dma_start(kT[:, e, p, :], k_gth[p, e, :, :])

        # ------------------------------ attention --------------------------
        with (
            tc.tile_pool(name="attn", bufs=3) as apool,
            tc.tile_pool(name="ptsb", bufs=6) as ptpool,
            tc.tile_pool(name="stats", bufs=2) as spool,
            tc.tile_pool(name="psT", bufs=2, space="PSUM") as psT,
            tc.tile_pool(name="psO", bufs=1, space="PSUM") as psO,
        ):
            psS = pmm
            for j in (7, 6, 5, 4, 3, 2, 1, 0):
                w = (j + 1) * P              # cols per parity block
                p_sb = apool.tile([P, 2, QL], BF16, tag="p")
                lsum = spool.tile([P, 2, NQT], F32, tag="lsum")
                for p in range(2):
                    off = 0
                    while off < w:
                        cw = min(512, w - off)
                        ps = psS.tile([P, cw], F32, tag="mm")
                        for e in range(ET):
                            nc.tensor.matmul(
                                ps, qT[:, e, j * P:(j + 1) * P],
                                kT[:, e, p, off:off + cw],
                                start=(e == 0), stop=(e == ET - 1))
                        if off + cw == w:
                            nc.vector.tensor_add(
                                ps[:, cw - P:cw], ps[:, cw - P:cw],
                                mask_sb[:, p * P:(p + 1) * P])
                        # 128-wide exp subtiles: each P^T transpose can start
                        # as soon as its own columns are exponentiated
                        for si in range(cw // P):
                            col = off + si * P
                            nc.scalar.activation(
                                p_sb[:, p, col:col + P],
                                ps[:, si * P:(si + 1) * P],
                                mybir.ActivationFunctionType.Exp,
                                accum_out=lsum[:, p, col // P:col // P + 1])
                        off += cw
                l2 = spool.tile([P, 2], F32, tag="l2")
                nc.vector.reduce_sum(l2, lsum[:, :, 0:j + 1],
                                     axis=mybir.AxisListType.X)
                l_ = spool.tile([P, 1], F32, tag="l")
                nc.vector.tensor_add(l_, l2[:, 0:1], l2[:, 1:2])
                linv = spool.tile([P, 1], F32, tag="linv")
                nc.vector.reciprocal(linv, l_)

                po = psO.tile([P, D], F32, tag="o")
                nk = 2 * (j + 1)
                for ki in range(nk):
                    p, k = ki % 2, ki // 2
                    pt_ps = psT.tile([P, P], BF16, tag="pt")
                    nc.tensor.transpose(pt_ps, p_sb[:, p, k * P:(k + 1) * P],
                                        ident)
                    pt = ptpool.tile([P, P], BF16, tag="ptsb")
                    nc.vector.tensor_copy(pt, pt_ps)
                    for c in range(D // 512):
                        nc.tensor.matmul(
                            po[:, c * 512:(c + 1) * 512], pt,
                            v[:, 2 * k + p, c * 512:(c + 1) * 512],
                            start=(ki == 0), stop=(ki == nk - 1))
                o_sb = apool.tile([P, D], F32, tag="o")
                for c in range(D // 512):
                    nc.vector.tensor_scalar_mul(
                        o_sb[:, c * 512:(c + 1) * 512],
                        po[:, c * 512:(c + 1) * 512], linv)
                nc.sync.dma_start(out[j * P:(j + 1) * P, :], o_sb)
        outer.__exit__(None, None, None)


_PROG = None


def _get_prog():
    global _PROG
    if _PROG is None:
        nc = bacc.Bacc("TRN2", target_bir_lowering=False, debug=False,
                       enable_asserts=False)
        x = nc.dram_tensor("x", (S, D), F32, kind="ExternalInput").ap()
        xq = nc.dram_tensor("xq", (QL, D), F32, kind="ExternalInput").ap()
        wq = nc.dram_tensor("wq", (D, D), F32, kind="ExternalInput").ap()
        wk = nc.dram_tensor("wk", (D, D), F32, kind="ExternalInput").ap()
        wv = nc.dram_tensor("wv", (D, D), F32, kind="ExternalInput").ap()
        mask = nc.dram_tensor("mask", (P, 256), F32, kind="ExternalInput").ap()
        out = nc.dram_tensor("out", (QL, D), F32, kind="ExternalOutput").ap()
        with tile.TileContext(nc) as tc:
            _body(tc, x, xq, wq, wk, wv, mask, out)
        nc.compile()
        _PROG = nc
    return _PROG


def _mask_np(h):
    r = np.arange(P)[:, None]
    c = np.arange(P)[None, :]
    tri = np.where(c <= r, 0.0, NEG).astype(np.float32)
    m = np.zeros((P, 256), np.float32)
    if h == 0:
        m[:, :P] = tri
        m[:, P:] = NEG
    else:
        m[:, P:] = tri
    return m


def _in_map_for_core(inputs, core):
    b, h = core // 2, core % 2
    xb = np.ascontiguousarray(np.asarray(inputs["x"], np.float32)[b])
    xqb = np.ascontiguousarray(xb.reshape(NQT, 2, P, D)[:, h].reshape(QL, D))
    return {
        "x": xb,
        "xq": xqb,
        "wq": np.ascontiguousarray(np.asarray(inputs["wq"], np.float32)),
        "wk": np.ascontiguousarray(np.asarray(inputs["wk"], np.float32)),
        "wv": np.ascontiguousarray(np.asarray(inputs["wv"], np.float32)),
        "mask": _mask_np(h),
    }


def _run(inputs, trace=False, tmpdir=None):
    nc = _get_prog()
    in_maps = [_in_map_for_core(inputs, c) for c in range(NCORES)]
    try:
        res = run_bass_kernel_spmd(nc, in_maps, core_ids=list(range(NCORES)),
                                   trace=trace, tmpdir=tmpdir)
    except Exception:
        # first execution of a fresh NEFF occasionally trips a transient
        # device error on this stack; one retry has always succeeded
        res = run_bass_kernel_spmd(nc, in_maps, core_ids=list(range(NCORES)),
                                   trace=trace, tmpdir=tmpdir)
    outf = np.empty((B, S, D), np.float32)
    for core in range(NCORES):
        b, h = core // 2, core % 2
        o = np.asarray(res.results[core]["out"], np.float32)
        outf[b].reshape(NQT, 2, P, D)[:, h] = o.reshape(NQT, P, D)
    return outf, res


def kernel(x, wq, wk, wv):
    outf, _ = _run({"x": x, "wq": wq, "wk": wk, "wv": wv}, trace=False)
    return outf
